# revision 1
# baseline (speedup 1.0000x reference)
"""Trainium2 Bass kernel for nn_Encoder: 6-layer post-LN transformer encoder.

Sharding: pure data-parallel over batch across 8 NeuronCores (2 sequences per
core), zero collectives. On-device layout is feature-major ([D on partitions,
tokens on free dim]) so every projection uses the stored weight directly as the
matmul stationary operand and per-feature biases are per-partition scalars.

Attention computes transposed scores [t, s] per head (K=64 matmuls on partition
halves), exponentiates without max-subtraction (scores are O(1) by
construction; masking is exp(s)*(1-m), exact since exp(-1e9) underflows to 0),
and contracts PV with col-tiled matmuls writing the two heads of a pair into
the two partition halves of one PSUM bank. Softmax denominators come from an
M=1 ones matmul; normalization is broadcast back over partitions with a K=1
outer product on the PE.

LayerNorm reduces over the feature (partition) axis with ones-matmuls
(E[x^2]-E[x]^2+eps), then applies (z*rstd)*g + (-g*mean*rstd + b) where the
per-token row factors are broadcast across partitions via K=1/K=2 PE outer
products and per-feature factors are per-partition scalars.
"""

import os
import sys

import numpy as np

sys.path.insert(0, "/opt/trn_rl_repo")

import concourse.bass as bass  # noqa: E402
import concourse.mybir as mybir  # noqa: E402
import concourse.tile as tile  # noqa: E402
from concourse import bacc  # noqa: E402
from concourse.bass_utils import run_bass_kernel_spmd  # noqa: E402
from concourse.masks import make_identity  # noqa: E402

# Problem constants (hardcoded per harness contract).
V, D, H, F = 32000, 768, 12, 3072
L = int(os.environ.get("ENC_LAYERS", "6"))
DN = D // H            # 64
B, S = 16, 512
NCORES = 8
BL = B // NCORES       # 2 sequences per core
T = BL * S             # 1024 tokens per core
P = 128
DT = D // P            # 6 feature tiles
TC = T // P            # 8 token chunks
SC = S // P            # 4 chunks per sequence
FT = F // P            # 24 ff tiles
NCH = 2                # T split into chunks of 512 for matmul free dim
CH = T // NCH          # 512
REPS = int(os.environ.get("ENC_REPS", "1"))  # timing: rerun layers in-NEFF
NODMAW = os.environ.get("ENC_NODMA_W", "0") == "1"  # debug: skip weight DMA
SKIP = set(os.environ.get("ENC_SKIP", "").split(","))  # debug: skip phases
EPS = 1e-5
FP32 = mybir.dt.float32
FP32R = mybir.dt.float32r
I32 = mybir.dt.int32

AF = mybir.ActivationFunctionType
OP = mybir.AluOpType

_PROGRAM_CACHE = {}


def _build_program():
    nc = bacc.Bacc("TRN2", target_bir_lowering=False, debug=False,
                   num_devices=NCORES)

    io = {}

    def inp(name, shape, dtype=FP32):
        io[name] = nc.declare_dram_parameter(name, list(shape), dtype,
                                             isOutput=False)

    inp("x_idx", [TC, P], I32)
    inp("emb", [V, D])
    inp("pe", [P, DT, S])
    inp("mmask", [P, BL, SC, S])
    inp("wq", [L, DT, DT, P, P], FP32R)   # [l, ktile, mtile, 128k, 128m]
    inp("wk", [L, DT, DT, P, P], FP32R)
    inp("wo", [L, DT, DT, P, P], FP32R)
    inp("wv", [L, DT, P, D], FP32R)       # [l, ktile, 128k, 768m]
    inp("w1", [L, FT, P, DT, P], FP32R)   # [l, mtile, 128k-part, ktile, 128m]
    inp("w2", [L, FT, P, D], FP32R)       # [l, ktile, 128k, 768m]
    inp("bq_c", [L, P, DT])
    inp("bk_c", [L, P, DT])
    inp("bv_r", [L, D])
    inp("bo_c", [L, P, DT])
    inp("b1_c", [L, P, FT])
    inp("b2_c", [L, P, DT])
    inp("g1_c", [L, P, DT])
    inp("g2_c", [L, P, DT])
    inp("gb1", [L, 2, D], FP32R)
    inp("gb2", [L, 2, D], FP32R)
    io["out"] = nc.declare_dram_parameter("out", [P, DT, T], FP32,
                                          isOutput=True)

    with tile.TileContext(nc) as tc:
        _emit(nc, tc, io)
    nc.compile()
    return nc


def _emit(nc, tc, io):
    from contextlib import ExitStack

    with ExitStack() as ctx:
        singles = ctx.enter_context(tc.tile_pool(name="singles", bufs=1))
        acts = ctx.enter_context(tc.tile_pool(name="acts", bufs=1))
        wpool = ctx.enter_context(tc.tile_pool(name="wpool", bufs=8))
        w1pool = ctx.enter_context(tc.tile_pool(name="w1pool", bufs=2))
        w2pool = ctx.enter_context(tc.tile_pool(name="w2pool", bufs=2))
        lw = ctx.enter_context(tc.tile_pool(name="lw", bufs=2))
        tmp = ctx.enter_context(tc.tile_pool(name="tmp", bufs=4))
        smalls = ctx.enter_context(tc.tile_pool(name="smalls", bufs=2))
        ps8 = ctx.enter_context(tc.tile_pool(name="ps8", bufs=8,
                                             space="PSUM"))

        # ---- persistent activations (feature-major unless noted) ----
        h = acts.tile([P, DT, T], FP32R)
        q = acts.tile([P, DT, T], FP32R)   # also holds attention output o
        k = acts.tile([P, DT, T], FP32R)
        v = acts.tile([P, TC, H, DN + 1], FP32R)  # token-major, +ones col
        o = q
        mm_sb = acts.tile([P, BL, SC, S], FP32)
        nc.sync.dma_start(mm_sb, io["mmask"][:])

        # ---- constants ----
        ident = singles.tile([P, P], FP32)
        make_identity(nc, ident)
        cst_f = singles.tile([P, 2], FP32)
        nc.vector.memset(cst_f[:, 0:1], 1.0)
        nc.vector.memset(cst_f[:, 1:2], 1.0 / D)
        cst_r = singles.tile([P, 2], FP32R)
        nc.vector.tensor_copy(cst_r, cst_f)
        ones_col = cst_r[:, 0:1]
        inv_d_col = cst_r[:, 1:2]
        row_f = singles.tile([1, CH], FP32)
        nc.vector.memset(row_f, 1.0)
        ones_row512 = singles.tile([1, CH], FP32R)
        nc.vector.tensor_copy(ones_row512, row_f)
        ones_row64 = ones_row512[:, 0:64]
        ones_row128 = ones_row512[:, 0:P]
        op64_f = singles.tile([65, 64], FP32)
        nc.vector.memset(op64_f[64:65, :], 1.0)
        ones_p64 = singles.tile([65, 64], FP32R)
        nc.vector.tensor_copy(ones_p64[64:65, :], op64_f[64:65, :])
        # ones column of v (written once; evictions only touch cols 0:DN)
        nc.vector.tensor_copy(v[:, :, :, DN],
                              cst_r[:, 0:1].to_broadcast((P, TC, H)))

        # ---- embedding gather + transpose to feature-major + positional ----
        with tc.tile_pool(name="embp", bufs=2) as embp:
            pe_sb = embp.tile([P, DT, S], FP32, bufs=1)
            nc.sync.dma_start(pe_sb, io["pe"][:])
            for c in range(TC):
                idx_t = embp.tile([P, 1], I32, tag="idx")
                nc.sync.dma_start(
                    idx_t, io["x_idx"][c].rearrange("(p o) -> p o", o=1))
                etok = embp.tile([P, D], FP32, tag="etok")
                nc.gpsimd.indirect_dma_start(
                    out=etok[:], out_offset=None, in_=io["emb"][:],
                    in_offset=bass.IndirectOffsetOnAxis(ap=idx_t[:, :1], axis=0))
                sc = c % SC  # position chunk within the sequence
                for ft in range(DT):
                    tp_ps = ps8.tile([P, P], FP32, tag="ps")
                    nc.tensor.transpose(tp_ps, etok[:, ft * P:(ft + 1) * P],
                                        ident)
                    nc.vector.tensor_tensor(
                        out=h[:, ft, c * P:(c + 1) * P], in0=tp_ps,
                        in1=pe_sb[:, ft, sc * P:(sc + 1) * P], op=OP.add)

        # ---- layers ----
        if REPS > 1:
            h0_save = nc.dram_tensor("h0_save", [P, DT, T], FP32R)
            nc.sync.dma_start(h0_save.ap(), h)
        for rep in range(REPS):
          if rep > 0:
            nc.sync.dma_start(h, h0_save.ap())
          for l in range(L):
            # per-layer bias / layernorm parameter tiles
            bq_t = lw.tile([P, DT], FP32, tag="bq")
            nc.sync.dma_start(bq_t, io["bq_c"][l])
            bk_t = lw.tile([P, DT], FP32, tag="bk")
            nc.sync.dma_start(bk_t, io["bk_c"][l])
            bo_t = lw.tile([P, DT], FP32, tag="bo")
            nc.sync.dma_start(bo_t, io["bo_c"][l])
            b1_t = lw.tile([P, FT], FP32, tag="b1")
            nc.sync.dma_start(b1_t, io["b1_c"][l])
            b2_t = lw.tile([P, DT], FP32, tag="b2")
            nc.sync.dma_start(b2_t, io["b2_c"][l])
            g1_t = lw.tile([P, DT], FP32, tag="g1")
            nc.sync.dma_start(g1_t, io["g1_c"][l])
            g2_t = lw.tile([P, DT], FP32, tag="g2")
            nc.sync.dma_start(g2_t, io["g2_c"][l])
            gneg1_t = lw.tile([1, D], FP32R, tag="gneg1", bufs=1)
            nc.sync.dma_start(gneg1_t, io["gb1"][l, 0:1, :])
            brow1_t = lw.tile([1, D], FP32R, tag="brow1", bufs=1)
            nc.sync.dma_start(brow1_t, io["gb1"][l, 1:2, :])
            gneg2_t = lw.tile([1, D], FP32R, tag="gneg2", bufs=1)
            nc.sync.dma_start(gneg2_t, io["gb2"][l, 0:1, :])
            brow2_t = lw.tile([1, D], FP32R, tag="brow2", bufs=1)
            nc.sync.dma_start(brow2_t, io["gb2"][l, 1:2, :])
            gb1_t = (gneg1_t, brow1_t, ones_row512)
            gb2_t = (gneg2_t, brow2_t, ones_row512)
            bv_t = lw.tile([P, D], FP32, tag="bv", bufs=1)
            bvl = io["bv_r"][l]
            nc.sync.dma_start(
                bv_t, bass.AP(tensor=bvl.tensor, offset=bvl.offset,
                              ap=[[0, P]] + list(bvl.ap)))

            # ---------- q/k projections ([P,P] weight blocks) ----------
            if "qkv" not in SKIP:
                for wname, bias_t, dst in (("wq", bq_t, q), ("wk", bk_t, k)):
                    for mt in range(DT):
                        for ch in range(NCH):
                            ps = ps8.tile([P, CH], FP32, tag="ps")
                            for kt in range(DT):
                                wt = wpool.tile([P, P], FP32R, tag="wblk")
                                if NODMAW:
                                    nc.sync.dma_start(wt[:, 0:1],
                                                      io[wname][l, kt, mt][:, 0:1])
                                else:
                                    nc.sync.dma_start(wt, io[wname][l, kt, mt])
                                nc.tensor.matmul(
                                    ps, lhsT=wt,
                                    rhs=h[:, kt, ch * CH:(ch + 1) * CH],
                                    start=(kt == 0), stop=(kt == DT - 1))
                            nc.vector.tensor_scalar(
                                out=dst[:, mt, ch * CH:(ch + 1) * CH], in0=ps,
                                scalar1=bias_t[:, mt:mt + 1], scalar2=None,
                                op0=OP.add)
                # v (token-major): out[t_chunk, features], ktile weights
                with tc.tile_pool(name="wvp", bufs=7) as wvp:
                    wts = []
                    for kt in range(DT):
                        wt = wvp.tile([P, D], FP32R, tag="wv")
                        if NODMAW:
                            nc.sync.dma_start(wt[:, 0:1], io["wv"][l, kt][:, 0:1])
                        else:
                            nc.sync.dma_start(wt, io["wv"][l, kt])
                        wts.append(wt)
                    HD = D // 2
                    for tch in range(TC):
                        for nh in range(2):
                            ps = ps8.tile([P, CH], FP32, tag="ps")
                            psn = ps[:, :HD]
                            for kt in range(DT):
                                nc.tensor.matmul(
                                    psn, lhsT=h[:, kt, tch * P:(tch + 1) * P],
                                    rhs=wts[kt][:, nh * HD:(nh + 1) * HD],
                                    start=(kt == 0), stop=(kt == DT - 1))
                            nc.vector.tensor_tensor(
                                out=v[:, tch, nh * (H // 2):(nh + 1) * (H // 2),
                                      0:DN],
                                in0=psn.rearrange("p (hh e) -> p hh e", e=DN),
                                in1=bv_t[:, nh * HD:(nh + 1) * HD].rearrange(
                                    "p (hh e) -> p hh e", e=DN),
                                op=OP.add)

            # ---------- attention ----------
            if "att" in SKIP:
                attp = None
            else:
             with tc.tile_pool(name="attp", bufs=3) as attp:
                for bb in range(BL):
                    for hp in range(DT):  # head pair: heads 2hp, 2hp+1
                        exs = []
                        for hh in range(2):
                            ex = attp.tile([P, SC, S], FP32R, tag="ex")
                            pr = slice(hh * 64, (hh + 1) * 64)
                            for tci in range(SC):
                                st = ps8.tile([P, S], FP32, tag="ps")
                                nc.tensor.matmul(
                                    st,
                                    lhsT=k[pr, hp, bb * S + tci * P:
                                           bb * S + (tci + 1) * P],
                                    rhs=q[pr, hp, bb * S:(bb + 1) * S],
                                    start=True, stop=True)
                                nc.scalar.activation(ex[:, tci, :], st,
                                                     AF.Exp)
                            eng = nc.vector if hh == 0 else nc.gpsimd
                            eng.tensor_tensor(out=ex[:], in0=ex[:],
                                              in1=mm_sb[:, bb], op=OP.mult)
                            exs.append(ex)
                        pvs = [ps8.tile([65, S], FP32, tag="ps",
                                           name=f"pv{i}") for i in range(2)]
                        for tci in range(SC):
                            tg = bb * SC + tci
                            for hh in range(2):
                                hd = 2 * hp + hh
                                nc.tensor.matmul(
                                    pvs[hh],
                                    lhsT=v[:, tg, hd, :],
                                    rhs=exs[hh][:, tci, :],
                                    start=(tci == 0), stop=(tci == SC - 1))
                        rc = smalls.tile([65, 2, S], FP32R, tag="rc")
                        with nc.allow_low_precision(
                                reason="fp32r softmax denominators"):
                            nc.vector.reciprocal(rc[64:65, 0, :],
                                                 pvs[0][64:65, :])
                            nc.vector.reciprocal(rc[64:65, 1, :],
                                                 pvs[1][64:65, :])
                        bcs = [ps8.tile([64, S], FP32, tag="ps",
                                           name=f"bc{i}") for i in range(2)]
                        nc.tensor.matmul(bcs[0], lhsT=ones_p64[64:65, :],
                                         rhs=rc[64:65, 0, :], start=True,
                                         stop=True)
                        nc.tensor.matmul(bcs[1], lhsT=ones_p64[64:65, :],
                                         rhs=rc[64:65, 1, :], start=True,
                                         stop=True)
                        # head even: normalize straight into o[0:64]
                        nc.vector.tensor_copy(
                            o[0:64, hp, bb * S:(bb + 1) * S], pvs[0][0:64, :])
                        nc.vector.tensor_tensor(
                            out=o[0:64, hp, bb * S:(bb + 1) * S],
                            in0=o[0:64, hp, bb * S:(bb + 1) * S], in1=bcs[0],
                            op=OP.mult)
                        # head odd: normalize at partitions 0-63, then
                        # DMA-shift into partitions 64-127 of o
                        ot = tmp.tile([64, S], FP32R, tag="scr")
                        nc.vector.tensor_copy(ot, pvs[1][0:64, :])
                        nc.vector.tensor_tensor(out=ot, in0=ot, in1=bcs[1],
                                                op=OP.mult)
                        nc.sync.dma_start(
                            o[64:128, hp, bb * S:(bb + 1) * S], ot)

            # ---------- Wo + residual, then LN1 ----------
            if "wo" not in SKIP:
                for mt in range(DT):
                    for ch in range(NCH):
                        ps = ps8.tile([P, CH], FP32, tag="ps")
                        for kt in range(DT):
                            wt = wpool.tile([P, P], FP32R, tag="wblk")
                            if NODMAW:
                                nc.sync.dma_start(wt[:, 0:1],
                                                  io["wo"][l, kt, mt][:, 0:1])
                            else:
                                nc.sync.dma_start(wt, io["wo"][l, kt, mt])
                            nc.tensor.matmul(
                                ps, lhsT=wt,
                                rhs=o[:, kt, ch * CH:(ch + 1) * CH],
                                start=(kt == 0), stop=(kt == DT - 1))
                        nc.vector.scalar_tensor_tensor(
                            out=h[:, mt, ch * CH:(ch + 1) * CH], in0=ps,
                            scalar=bo_t[:, mt:mt + 1],
                            in1=h[:, mt, ch * CH:(ch + 1) * CH],
                            op0=OP.add, op1=OP.add)
            if "ln" not in SKIP:
                _layernorm(nc, tc, h, g1_t, gb1_t, ones_row128, inv_d_col,
                           tmp, smalls, ps8)

            # ---------- FFN + residual, then LN2 ----------
            if "ffn" not in SKIP:
                for ch in range(NCH):
                    accs = [ps8.tile([P, CH], FP32, tag="ps",
                                        name=f"acc{i}")
                            for i in range(DT)]
                    for m in range(FT):
                        w1t = w1pool.tile([P, DT, P], FP32R, tag="w1")
                        w2t = w2pool.tile([P, D], FP32R, tag="w2")
                        if NODMAW:
                            nc.sync.dma_start(w1t[:, :, 0:1],
                                              io["w1"][l, m][:, :, 0:1])
                            nc.sync.dma_start(w2t[:, 0:1],
                                              io["w2"][l, m][:, 0:1])
                        else:
                            nc.sync.dma_start(w1t, io["w1"][l, m])
                            nc.sync.dma_start(w2t, io["w2"][l, m])
                        ps = ps8.tile([P, CH], FP32, tag="ps")
                        for kt in range(DT):
                            nc.tensor.matmul(
                                ps, lhsT=w1t[:, kt, :],
                                rhs=h[:, kt, ch * CH:(ch + 1) * CH],
                                start=(kt == 0), stop=(kt == DT - 1))
                        ff_sb = tmp.tile([P, CH], FP32R, tag="scr")
                        nc.scalar.activation(ff_sb, ps, AF.Relu,
                                             bias=b1_t[:, m:m + 1])
                        for mt in range(DT):
                            nc.tensor.matmul(
                                accs[mt], lhsT=w2t[:, mt * P:(mt + 1) * P],
                                rhs=ff_sb, start=(m == 0), stop=(m == FT - 1))
                    for mt in range(DT):
                        nc.vector.scalar_tensor_tensor(
                            out=h[:, mt, ch * CH:(ch + 1) * CH], in0=accs[mt],
                            scalar=b2_t[:, mt:mt + 1],
                            in1=h[:, mt, ch * CH:(ch + 1) * CH],
                            op0=OP.add, op1=OP.add)
            if "ln" not in SKIP:
                _layernorm(nc, tc, h, g2_t, gb2_t, ones_row128, inv_d_col,
                           tmp, smalls, ps8)

        nc.sync.dma_start(io["out"][:], h[:].bitcast(FP32))


def _layernorm(nc, tc, h, g_t, gb_t, ones_row128, inv_d_col, tmp, smalls,
               ps8):
    """In-place LayerNorm over the feature (partition) axis of h [P, DT, T]."""
    if True:
        for ch in range(NCH):
            chs = slice(ch * CH, (ch + 1) * CH)
            mean_ps = ps8.tile([P, CH], FP32, tag="ps")
            msq_ps = ps8.tile([P, CH], FP32, tag="ps")
            for mt in range(DT):
                sq = tmp.tile([P, CH], FP32R, tag="scr")
                nc.scalar.activation(sq, h[:, mt, chs], AF.Square)
                nc.tensor.matmul(mean_ps[0:1, :], lhsT=inv_d_col,
                                 rhs=h[:, mt, chs], start=(mt == 0),
                                 stop=(mt == DT - 1))
                nc.tensor.matmul(msq_ps[0:1, :], lhsT=inv_d_col, rhs=sq,
                                 start=(mt == 0), stop=(mt == DT - 1))
            sqm = smalls.tile([1, CH], FP32, tag="s")
            nc.scalar.activation(sqm, mean_ps[0:1, :], AF.Square)
            var = smalls.tile([1, CH], FP32, tag="s")
            nc.vector.scalar_tensor_tensor(out=var, in0=msq_ps[0:1, :],
                                           scalar=EPS, in1=sqm, op0=OP.add,
                                           op1=OP.subtract)
            lnv = smalls.tile([1, CH], FP32, tag="s")
            nc.scalar.activation(lnv, var, AF.Ln)
            mr = smalls.tile([1, CH], FP32R, tag="s")
            nc.scalar.activation(mr, lnv, AF.Exp, scale=-0.5)  # rstd
            mmr = smalls.tile([1, CH], FP32R, tag="s")
            nc.vector.tensor_tensor(out=mmr, in0=mean_ps[0:1, :], in1=mr,
                                    op=OP.mult)            # mean*rstd
            rstd_b = ps8.tile([P, CH], FP32, tag="ps")
            nc.tensor.matmul(rstd_b, lhsT=ones_row128, rhs=mr,
                             start=True, stop=True)
            gneg_t, brow_t, ones_row512 = gb_t
            for mt in range(DT):
                c2 = ps8.tile([P, CH], FP32, tag="ps")
                nc.tensor.matmul(c2, lhsT=gneg_t[:, mt * P:(mt + 1) * P],
                                 rhs=mmr, start=True, stop=False)
                nc.tensor.matmul(c2, lhsT=brow_t[:, mt * P:(mt + 1) * P],
                                 rhs=ones_row512, start=False, stop=True)
                t2 = tmp.tile([P, CH], FP32, tag="scr")
                nc.vector.tensor_tensor(out=t2, in0=h[:, mt, chs], in1=rstd_b,
                                        op=OP.mult)
                nc.vector.scalar_tensor_tensor(
                    out=h[:, mt, chs], in0=t2, scalar=g_t[:, mt:mt + 1],
                    in1=c2, op0=OP.mult, op1=OP.add)


# ---------------- host side ----------------

def _pos_encoding_np():
    pos = np.arange(S, dtype=np.float32)[:, None]
    i = np.arange(D // 2, dtype=np.float32)[None, :]
    denom_s = np.power(np.float32(10000.0), (2.0 * i / D).astype(np.float32))
    denom_c = np.power(np.float32(10000.0),
                       (2.0 * (i + 1.0) / D).astype(np.float32))
    pe = np.zeros((S, D), np.float32)
    pe[:, 0::2] = np.sin(pos / denom_s)
    pe[:, 1::2] = np.cos(pos / denom_c)
    return pe  # [S, D]


def _prep_shared(emb, Wq, bq, Wk, bk, Wv, bv, Wo, bo, W1, b1, W2, b2,
                 g1, be1, g2, be2):
    f32 = np.float32
    scale = f32(1.0 / np.sqrt(DN))

    def cols(a, nt):  # [L, nt*128] -> [L, 128, nt]
        return np.ascontiguousarray(
            np.asarray(a).reshape(L, nt, P).transpose(0, 2, 1)).astype(f32)

    def blocks(a):  # [L, D, D] -> [L, DT, DT, P, P] (ktile, mtile blocks)
        return np.ascontiguousarray(
            a.reshape(L, DT, P, DT, P).transpose(0, 1, 3, 2, 4)).astype(f32)

    Wq, Wk, Wv, Wo = (np.asarray(a)[:L] for a in (Wq, Wk, Wv, Wo))
    W1, W2 = np.asarray(W1)[:L], np.asarray(W2)[:L]
    bq, bk, bv, bo = (np.asarray(a)[:L] for a in (bq, bk, bv, bo))
    b1, b2 = np.asarray(b1)[:L], np.asarray(b2)[:L]
    g1, be1, g2, be2 = (np.asarray(a)[:L] for a in (g1, be1, g2, be2))

    wq_h = blocks(Wq.transpose(0, 2, 1, 3).reshape(L, D, D) * scale)
    wk_h = blocks(Wk.transpose(0, 2, 1, 3).reshape(L, D, D))
    wo_h = blocks(Wo.astype(f32))
    wv_h = np.ascontiguousarray(
        Wv.transpose(0, 2, 1, 3).reshape(L, DT, P, D)).astype(f32)
    w1_h = np.ascontiguousarray(
        W1.reshape(L, DT, P, FT, P).transpose(0, 3, 2, 1, 4)).astype(f32)
    w2_h = np.ascontiguousarray(W2.reshape(L, FT, P, D)).astype(f32)

    pe_np = _pos_encoding_np()  # [S, D]
    pe_h = np.ascontiguousarray(
        pe_np.T.reshape(DT, P, S).transpose(1, 0, 2)).astype(f32)

    return dict(
        emb=np.ascontiguousarray(emb).astype(f32),
        pe=pe_h,
        wq=wq_h, wk=wk_h, wv=wv_h, wo=wo_h, w1=w1_h, w2=w2_h,
        bq_c=cols(bq.reshape(L, D) * scale, DT),
        bk_c=cols(bk.reshape(L, D), DT),
        bv_r=np.ascontiguousarray(bv.reshape(L, D)).astype(f32),
        bo_c=cols(bo, DT),
        b1_c=cols(b1, FT),
        b2_c=cols(b2, DT),
        g1_c=cols(g1, DT),
        g2_c=cols(g2, DT),
        gb1=np.ascontiguousarray(np.stack([-g1, be1], axis=1)).astype(f32),
        gb2=np.ascontiguousarray(np.stack([-g2, be2], axis=1)).astype(f32),
    )


def kernel(x, padding_mask, emb, Wq, bq, Wk, bk, Wv, bv, Wo, bo,
           W1, b1, W2, b2, g1, be1, g2, be2):
    if "nc" not in _PROGRAM_CACHE:
        _PROGRAM_CACHE["nc"] = _build_program()
    nc = _PROGRAM_CACHE["nc"]

    shared = _prep_shared(emb, Wq, bq, Wk, bk, Wv, bv, Wo, bo, W1, b1, W2, b2,
                          g1, be1, g2, be2)

    x_i = np.asarray(x).astype(np.int32)
    mask_f = 1.0 - np.asarray(padding_mask).astype(np.float32)

    in_maps = []
    for c in range(NCORES):
        xs = x_i[c * BL:(c + 1) * BL].reshape(T)             # [1024]
        ms = mask_f[c * BL:(c + 1) * BL]                     # [2, 512, 512]
        # mmask[p, b, tci, s] = (1-mask)[b, s, tci*128+p]
        mt = np.ascontiguousarray(
            ms.transpose(0, 2, 1).reshape(BL, SC, P, S).transpose(2, 0, 1, 3))
        m = dict(shared)
        m["x_idx"] = np.ascontiguousarray(xs.reshape(TC, P))
        m["mmask"] = mt
        in_maps.append(m)

    res = run_bass_kernel_spmd(nc, in_maps, core_ids=list(range(NCORES)))

    outs = []
    for c in range(NCORES):
        oc = res.results[c]["out"]                    # [P, DT, T]
        hc = oc.transpose(2, 1, 0).reshape(T, D)      # [T, D]
        outs.append(hc.reshape(BL, S, D))
    return np.concatenate(outs, axis=0).astype(np.float32)


if __name__ == "__main__":
    pass



# revision 23
# speedup vs baseline: 25.2141x; 25.2141x over previous
"""Trainium2 Bass kernel for nn_Encoder: 6-layer post-LN transformer encoder.

Sharding: pure data-parallel over batch across 8 NeuronCores (2 sequences per
core), zero collectives. On-device layout is feature-major ([D on partitions,
tokens on free dim]) so every projection uses the stored weight directly as the
matmul stationary operand and per-feature biases are per-partition scalars.

Numerics: bf16 activations and weights, fp32 PSUM accumulation, fp32
layernorm statistics / softmax denominators. Weights are loaded as one
contiguous per-layer slab per group (qkvo: 4.7MB, ffn: 9.4MB) so DMA runs at
near-peak HBM bandwidth and the PE never waits on small descriptor-bound
transfers.

Attention computes transposed scores [t, s] per head (K=64 matmuls on
partition halves), exponentiates without max-subtraction (scores are O(1) by
construction; masking is exp(s)*(1-m), exact since exp(-1e9) underflows to 0),
and contracts PV with col-tiled matmuls. Softmax denominators come from a
ones-column appended to v; normalization is broadcast back over partitions
with a K=1 outer product on the PE.

LayerNorm reduces over the feature (partition) axis with ones-matmuls
(E[x^2]-E[x]^2+eps), then applies (z*rstd)*g + (-g*mean*rstd + b) where the
per-token row factors are broadcast across partitions via K=1/K=2 PE outer
products and per-feature factors are per-partition scalars.
"""

import os
import sys

import numpy as np

sys.path.insert(0, "/opt/trn_rl_repo")

import concourse.bass as bass  # noqa: E402
import concourse.mybir as mybir  # noqa: E402
import concourse.tile as tile  # noqa: E402
from concourse import bacc  # noqa: E402
from concourse.bass_utils import run_bass_kernel_spmd  # noqa: E402
from concourse.masks import make_identity  # noqa: E402

# Problem constants (hardcoded per harness contract).
V, D, H, F = 32000, 768, 12, 3072
L = int(os.environ.get("ENC_LAYERS", "6"))
DN = D // H            # 64
B, S = 16, 512
NCORES = 8
BL = B // NCORES       # 2 sequences per core
T = BL * S             # 1024 tokens per core
P = 128
DT = D // P            # 6 feature tiles
TC = T // P            # 8 token chunks
SC = S // P            # 4 chunks per sequence
FT = F // P            # 24 ff tiles
NCH = 2                # T split into chunks of 512 for matmul free dim
CH = T // NCH          # 512
EPS = 1e-5
FP32 = mybir.dt.float32
FP32R = mybir.dt.float32r
BF16 = mybir.dt.bfloat16
I32 = mybir.dt.int32
NPBF16 = mybir.dt.np(BF16)

AF = mybir.ActivationFunctionType
OP = mybir.AluOpType

_PROGRAM_CACHE = {}


class _single_act_table:
    """During compile, restrict the act-table chooser so every activation
    function we use (Exp/Ln/Relu/Square/...) resolves to the one table set
    that contains them all (natural_log_exp_and_others). Otherwise the pass
    alternates tables (Exp -> set 0, Ln -> set 5) and each swap costs a
    2.6us table load that stalls the PE mid-layer. Indices of the table
    list are preserved, so the emitted act_func_set_id stays a valid index
    into act_info.json. Restores the original resolver on exit."""

    TARGET = "natural_log_exp_and_others"

    def __enter__(self):
        import concourse.hw_specs as hw_specs
        self._orig = hw_specs.get_activation_tables
        orig = self._orig

        @__import__("functools").cache
        def patched(module_arch):
            tables = dict(orig(module_arch))
            if self.TARGET in tables:
                shared = tables[self.TARGET]
                tables = {
                    name: (s if name == self.TARGET else s - shared)
                    for name, s in tables.items()
                }
            return tables

        hw_specs.get_activation_tables = patched
        import concourse.bacc as bacc_mod
        bacc_mod.get_activation_tables = patched
        return self

    def __exit__(self, *exc):
        import concourse.hw_specs as hw_specs
        import concourse.bacc as bacc_mod
        hw_specs.get_activation_tables = self._orig
        bacc_mod.get_activation_tables = self._orig
        return False


def _build_program(reps=1):
    nc = bacc.Bacc("TRN2", target_bir_lowering=False, debug=False,
                   num_devices=NCORES)

    io = {}

    def inp(name, shape, dtype=FP32):
        io[name] = nc.declare_dram_parameter(name, list(shape), dtype,
                                             isOutput=False)

    inp("x_idx", [TC, P], I32)
    inp("emb", [V, D], BF16)
    inp("pe", [P, DT, S], BF16)
    inp("mmask", [P, BL, SC, S], BF16)
    # per-layer weight slabs, contiguous per partition line:
    # wqkvo[l, p, 0..2, kt, mt, m] = Wq/Wk/Wo blocks; [l, p, 3, kt, :] = Wv row
    inp("wqkvo", [L, P, 4, DT, DT * P], BF16)
    # wffn[l, p, m, 0:DT*P] = W1[l, :, m-tile] blocks (kt, mm);
    # wffn[l, p, m, DT*P:] = W2[l, m*128+p, :]
    inp("wffn", [L, P, FT, DT * P + D], BF16)
    inp("bq_c", [L, P, DT])
    inp("bk_c", [L, P, DT])
    inp("bv_r", [L, D])
    inp("bo_c", [L, P, DT])
    inp("b1_c", [L, P, FT])
    inp("b2_c", [L, P, DT])
    inp("g1_c", [L, P, DT])
    inp("g2_c", [L, P, DT])
    inp("ng1_c", [L, P, DT])   # -g1
    inp("ng2_c", [L, P, DT])
    inp("be1_c", [L, P, DT])   # beta
    inp("be2_c", [L, P, DT])
    io["out"] = nc.declare_dram_parameter("out", [P, DT, T], BF16,
                                          isOutput=True)

    with tile.TileContext(nc) as tc:
        _emit(nc, tc, io, reps)
    with _single_act_table():
        nc.compile()
    return nc


def _emit(nc, tc, io, reps):
    from contextlib import ExitStack

    with ExitStack() as ctx:
        singles = ctx.enter_context(tc.tile_pool(name="singles", bufs=1))
        acts = ctx.enter_context(tc.tile_pool(name="acts", bufs=1))
        wqkvop = ctx.enter_context(tc.tile_pool(name="wqkvop", bufs=1))
        wffnp = ctx.enter_context(tc.tile_pool(name="wffnp", bufs=1))
        lw = ctx.enter_context(tc.tile_pool(name="lw", bufs=2))
        tmp = ctx.enter_context(tc.tile_pool(name="tmp", bufs=4))
        smalls = ctx.enter_context(tc.tile_pool(name="smalls", bufs=2))
        ps8 = ctx.enter_context(tc.tile_pool(name="ps8", bufs=8,
                                             space="PSUM"))

        # ---- persistent activations (feature-major unless noted) ----
        h = acts.tile([P, DT, T], BF16)
        q = acts.tile([P, DT, T], BF16)   # also holds attention output o
        k = acts.tile([P, DT, T], BF16)
        v = acts.tile([P, TC, H, DN + 1], BF16)  # token-major, +ones col
        o = q
        mm_sb = acts.tile([P, BL, SC, S], BF16)
        nc.sync.dma_start(mm_sb, io["mmask"][:])

        # ---- constants (memset only supports 32-bit dtypes; copy-convert) ----
        ident_f = singles.tile([P, P], FP32)
        make_identity(nc, ident_f)
        ident = singles.tile([P, P], BF16)
        nc.vector.tensor_copy(ident, ident_f)
        cst_f = singles.tile([P, 2], FP32)
        nc.vector.memset(cst_f[:, 0:1], 1.0)
        nc.vector.memset(cst_f[:, 1:2], EPS)
        eps_t = cst_f[0:1, 1:2]
        ones_col = singles.tile([P, 1], BF16)
        nc.vector.tensor_copy(ones_col, cst_f[:, 0:1])
        row_f = singles.tile([1, P], FP32)
        nc.vector.memset(row_f, 1.0)
        ones_row128 = singles.tile([1, P], FP32R)
        nc.vector.tensor_copy(ones_row128, row_f)
        op64_f = singles.tile([65, 64], FP32)
        nc.vector.memset(op64_f[64:65, :], 1.0)
        ones_p64 = singles.tile([65, 64], BF16)
        nc.vector.tensor_copy(ones_p64[64:65, :], op64_f[64:65, :])
        # ones column of v (written once; evictions only touch cols 0:DN)
        nc.vector.tensor_copy(v[:, :, :, DN],
                              cst_f[:, 0:1].to_broadcast((P, TC, H)))

        # ---- embedding gather + transpose to feature-major + positional ----
        with tc.tile_pool(name="embp", bufs=2) as embp:
            pe_sb = embp.tile([P, DT, S], BF16, bufs=1)
            nc.sync.dma_start(pe_sb, io["pe"][:])
            for c in range(TC):
                idx_t = embp.tile([P, 1], I32, tag="idx")
                nc.sync.dma_start(
                    idx_t, io["x_idx"][c].rearrange("(p o) -> p o", o=1))
                etok = embp.tile([P, D], BF16, tag="etok")
                nc.gpsimd.indirect_dma_start(
                    out=etok[:], out_offset=None, in_=io["emb"][:],
                    in_offset=bass.IndirectOffsetOnAxis(ap=idx_t[:, :1], axis=0))
                sc = c % SC  # position chunk within the sequence
                for ft in range(DT):
                    tp_ps = ps8.tile([P, P], BF16, tag="ps")
                    nc.tensor.transpose(tp_ps, etok[:, ft * P:(ft + 1) * P],
                                        ident)
                    nc.vector.tensor_tensor(
                        out=h[:, ft, c * P:(c + 1) * P], in0=tp_ps,
                        in1=pe_sb[:, ft, sc * P:(sc + 1) * P], op=OP.add)

        # ---- layers ----
        if reps > 1:
            h0_save = nc.dram_tensor("h0_save", [P, DT, T], BF16)
            nc.sync.dma_start(h0_save.ap(), h)
        for rep in range(reps):
          if rep > 0:
            nc.sync.dma_start(h, h0_save.ap())
          for l in range(L):
            # per-layer weight slabs (single large DMAs)
            wqkvo = wqkvop.tile([P, 4, DT, DT * P], BF16, tag="wqkvo")
            nc.sync.dma_start(wqkvo, io["wqkvo"][l])
            wffn = wffnp.tile([P, FT, DT * P + D], BF16, tag="wffn")
            nc.sync.dma_start(wffn, io["wffn"][l])

            def wblk(g, kt, mt):  # [128k, 128m] block of Wq/Wk/Wo
                return wqkvo[:, g, kt, mt * P:(mt + 1) * P]

            # per-layer bias / layernorm parameter tiles
            bq_t = lw.tile([P, DT], FP32, tag="bq")
            nc.sync.dma_start(bq_t, io["bq_c"][l])
            bk_t = lw.tile([P, DT], FP32, tag="bk")
            nc.sync.dma_start(bk_t, io["bk_c"][l])
            bo_t = lw.tile([P, DT], FP32, tag="bo")
            nc.sync.dma_start(bo_t, io["bo_c"][l])
            b1_t = lw.tile([P, FT], FP32, tag="b1")
            nc.sync.dma_start(b1_t, io["b1_c"][l])
            b2_t = lw.tile([P, DT], FP32, tag="b2")
            nc.sync.dma_start(b2_t, io["b2_c"][l])
            g1_t = lw.tile([P, DT], FP32, tag="g1")
            nc.sync.dma_start(g1_t, io["g1_c"][l])
            g2_t = lw.tile([P, DT], FP32, tag="g2")
            nc.sync.dma_start(g2_t, io["g2_c"][l])
            ng1_t = lw.tile([P, DT], FP32, tag="ng1")
            nc.sync.dma_start(ng1_t, io["ng1_c"][l])
            ng2_t = lw.tile([P, DT], FP32, tag="ng2")
            nc.sync.dma_start(ng2_t, io["ng2_c"][l])
            be1_t = lw.tile([P, DT], FP32, tag="be1")
            nc.sync.dma_start(be1_t, io["be1_c"][l])
            be2_t = lw.tile([P, DT], FP32, tag="be2")
            nc.sync.dma_start(be2_t, io["be2_c"][l])
            gb1_t = (ng1_t, be1_t)
            gb2_t = (ng2_t, be2_t)
            bv_t = lw.tile([P, D], FP32, tag="bv", bufs=1)
            bvl = io["bv_r"][l]
            nc.sync.dma_start(
                bv_t, bass.AP(tensor=bvl.tensor, offset=bvl.offset,
                              ap=[[0, P]] + list(bvl.ap)))

            # ---------- q/k projections ([P,P] weight blocks) ----------
            for g, bias_t, dst in ((0, bq_t, q), (1, bk_t, k)):
                for mt in range(DT):
                    for ch in range(NCH):
                        ps = ps8.tile([P, CH], FP32, tag="ps")
                        for kt in range(DT):
                            nc.tensor.matmul(
                                ps, lhsT=wblk(g, kt, mt),
                                rhs=h[:, kt, ch * CH:(ch + 1) * CH],
                                start=(kt == 0), stop=(kt == DT - 1))
                        nc.vector.tensor_scalar(
                            out=dst[:, mt, ch * CH:(ch + 1) * CH], in0=ps,
                            scalar1=bias_t[:, mt:mt + 1], scalar2=None,
                            op0=OP.add)
            # v (token-major): out[t_chunk, features], ktile weights
            HD = D // 2
            for tch in range(TC):
                for nh in range(2):
                    ps = ps8.tile([P, CH], FP32, tag="ps")
                    psn = ps[:, :HD]
                    for kt in range(DT):
                        nc.tensor.matmul(
                            psn, lhsT=h[:, kt, tch * P:(tch + 1) * P],
                            rhs=wqkvo[:, 3, kt, nh * HD:(nh + 1) * HD],
                            start=(kt == 0), stop=(kt == DT - 1))
                    nc.vector.tensor_tensor(
                        out=v[:, tch, nh * (H // 2):(nh + 1) * (H // 2),
                              0:DN],
                        in0=psn.rearrange("p (hh e) -> p hh e", e=DN),
                        in1=bv_t[:, nh * HD:(nh + 1) * HD].rearrange(
                            "p (hh e) -> p hh e", e=DN),
                        op=OP.add)

            # ---------- attention ----------
            with tc.tile_pool(name="attp", bufs=3) as attp:
                for bb in range(BL):
                    for hp in range(DT):  # head pair: heads 2hp, 2hp+1
                        exs = []
                        for hh in range(2):
                            ex = attp.tile([P, SC, S], BF16, tag="ex")
                            pr = slice(hh * 64, (hh + 1) * 64)
                            for tci in range(SC):
                                st = ps8.tile([P, S], FP32, tag="ps")
                                nc.tensor.matmul(
                                    st,
                                    lhsT=k[pr, hp, bb * S + tci * P:
                                           bb * S + (tci + 1) * P],
                                    rhs=q[pr, hp, bb * S:(bb + 1) * S],
                                    start=True, stop=True)
                                nc.scalar.activation(ex[:, tci, :], st,
                                                     AF.Exp)
                                eng = nc.vector if hh == 0 else nc.gpsimd
                                eng.tensor_tensor(
                                    out=ex[:, tci, :], in0=ex[:, tci, :],
                                    in1=mm_sb[:, bb, tci, :], op=OP.mult)
                            exs.append(ex)
                        pvs = [ps8.tile([65, S], FP32, tag="ps",
                                           name=f"pv{i}") for i in range(2)]
                        for tci in range(SC):
                            tg = bb * SC + tci
                            for hh in range(2):
                                hd = 2 * hp + hh
                                nc.tensor.matmul(
                                    pvs[hh],
                                    lhsT=v[:, tg, hd, :],
                                    rhs=exs[hh][:, tci, :],
                                    start=(tci == 0), stop=(tci == SC - 1))
                        rc = smalls.tile([65, 2, S], BF16, tag="rc")
                        with nc.allow_low_precision(
                                reason="fp32r softmax denominators"):
                            nc.vector.reciprocal(rc[64:65, 0, :],
                                                 pvs[0][64:65, :])
                            nc.vector.reciprocal(rc[64:65, 1, :],
                                                 pvs[1][64:65, :])
                        bcs = [ps8.tile([64, S], FP32, tag="ps",
                                           name=f"bc{i}") for i in range(2)]
                        nc.tensor.matmul(bcs[0], lhsT=ones_p64[64:65, :],
                                         rhs=rc[64:65, 0, :], start=True,
                                         stop=True)
                        nc.tensor.matmul(bcs[1], lhsT=ones_p64[64:65, :],
                                         rhs=rc[64:65, 1, :], start=True,
                                         stop=True)
                        # head even: normalize straight into o[0:64]
                        nc.vector.tensor_copy(
                            o[0:64, hp, bb * S:(bb + 1) * S], pvs[0][0:64, :])
                        nc.vector.tensor_tensor(
                            out=o[0:64, hp, bb * S:(bb + 1) * S],
                            in0=o[0:64, hp, bb * S:(bb + 1) * S], in1=bcs[0],
                            op=OP.mult)
                        # head odd: normalize at partitions 0-63, then
                        # DMA-shift into partitions 64-127 of o
                        ot = tmp.tile([64, S], BF16, tag="ot")
                        nc.vector.tensor_copy(ot, pvs[1][0:64, :])
                        nc.vector.tensor_tensor(out=ot, in0=ot, in1=bcs[1],
                                                op=OP.mult)
                        nc.sync.dma_start(
                            o[64:128, hp, bb * S:(bb + 1) * S], ot)

            # ---------- Wo + residual, then LN1 ----------
            for mt in range(DT):
                for ch in range(NCH):
                    ps = ps8.tile([P, CH], FP32, tag="ps")
                    for kt in range(DT):
                        nc.tensor.matmul(
                            ps, lhsT=wblk(2, kt, mt),
                            rhs=o[:, kt, ch * CH:(ch + 1) * CH],
                            start=(kt == 0), stop=(kt == DT - 1))
                    nc.vector.scalar_tensor_tensor(
                        out=h[:, mt, ch * CH:(ch + 1) * CH], in0=ps,
                        scalar=bo_t[:, mt:mt + 1],
                        in1=h[:, mt, ch * CH:(ch + 1) * CH],
                        op0=OP.add, op1=OP.add)
            _layernorm(nc, tc, h, g1_t, gb1_t, ones_row128, ones_col, eps_t,
                       tmp, smalls, ps8)

            # ---------- FFN + residual, then LN2 ----------
            for ch in range(NCH):
                accs = [ps8.tile([P, CH], FP32, tag="ps",
                                    name=f"acc{i}")
                        for i in range(DT)]
                for m in range(FT):
                    ps = ps8.tile([P, CH], FP32, tag="ps")
                    for kt in range(DT):
                        nc.tensor.matmul(
                            ps, lhsT=wffn[:, m, kt * P:(kt + 1) * P],
                            rhs=h[:, kt, ch * CH:(ch + 1) * CH],
                            start=(kt == 0), stop=(kt == DT - 1))
                    ff_sb = tmp.tile([P, CH], BF16, tag="scr")
                    nc.scalar.activation(ff_sb, ps, AF.Relu,
                                         bias=b1_t[:, m:m + 1])
                    for mt in range(DT):
                        nc.tensor.matmul(
                            accs[mt],
                            lhsT=wffn[:, m, DT * P + mt * P:
                                      DT * P + (mt + 1) * P],
                            rhs=ff_sb, start=(m == 0), stop=(m == FT - 1))
                for mt in range(DT):
                    nc.vector.scalar_tensor_tensor(
                        out=h[:, mt, ch * CH:(ch + 1) * CH], in0=accs[mt],
                        scalar=b2_t[:, mt:mt + 1],
                        in1=h[:, mt, ch * CH:(ch + 1) * CH],
                        op0=OP.add, op1=OP.add)
            _layernorm(nc, tc, h, g2_t, gb2_t, ones_row128, ones_col, eps_t,
                       tmp, smalls, ps8)

        nc.sync.dma_start(io["out"][:], h)


def _layernorm(nc, tc, h, g_t, gb_t, ones_row128, ones_col, eps_t, tmp,
               smalls, ps8):
    """In-place LayerNorm over the feature (partition) axis of h [P, DT, T]."""
    inv_d = float(1.0 / D)
    for ch in range(NCH):
        chs = slice(ch * CH, (ch + 1) * CH)
        sum_ps = ps8.tile([P, CH], FP32, tag="ps")   # sum(h) over features
        ssq_ps = ps8.tile([P, CH], FP32, tag="ps")   # sum(h^2)
        for mt in range(DT):
            sq = tmp.tile([P, CH], BF16, tag="scr")
            nc.scalar.activation(sq, h[:, mt, chs], AF.Square)
            nc.tensor.matmul(sum_ps[0:1, :], lhsT=ones_col,
                             rhs=h[:, mt, chs], start=(mt == 0),
                             stop=(mt == DT - 1))
            nc.tensor.matmul(ssq_ps[0:1, :], lhsT=ones_col, rhs=sq,
                             start=(mt == 0), stop=(mt == DT - 1))
        sqm = smalls.tile([1, CH], FP32, tag="s")
        nc.scalar.activation(sqm, sum_ps[0:1, :], AF.Square,
                             scale=inv_d)            # mean^2
        var = smalls.tile([1, CH], FP32, tag="s")    # E[x^2]-mean^2 (no eps)
        nc.vector.scalar_tensor_tensor(out=var, in0=ssq_ps[0:1, :],
                                       scalar=inv_d, in1=sqm, op0=OP.mult,
                                       op1=OP.subtract)
        lnv = smalls.tile([1, CH], FP32, tag="s")
        nc.scalar.activation(lnv, var, AF.Ln, bias=eps_t[0:1, 0:1])
        mr = smalls.tile([1, CH], FP32R, tag="s")
        nc.scalar.activation(mr, lnv, AF.Exp, scale=-0.5)  # rstd
        mmr = smalls.tile([1, CH], FP32R, tag="s")
        nc.vector.scalar_tensor_tensor(out=mmr, in0=sum_ps[0:1, :],
                                       scalar=inv_d, in1=mr,
                                       op0=OP.mult, op1=OP.mult)  # mean*rstd
        # broadcast rstd / mean*rstd across partitions via K=1 matmuls
        rstd_b = ps8.tile([P, CH], FP32, tag="ps")
        nc.tensor.matmul(rstd_b, lhsT=ones_row128, rhs=mr,
                         start=True, stop=True)
        mmr_bp = ps8.tile([P, CH], FP32, tag="ps")
        nc.tensor.matmul(mmr_bp, lhsT=ones_row128, rhs=mmr,
                         start=True, stop=True)
        mmr_b = mmr_bp
        ng_t, be_t = gb_t
        for mt in range(DT):
            # c2 = (-g) * (mean*rstd) + beta   (per-partition scalars)
            c2 = tmp.tile([P, CH], FP32, tag="c2", bufs=2)
            nc.vector.tensor_scalar(
                out=c2, in0=mmr_b, scalar1=ng_t[:, mt:mt + 1],
                scalar2=be_t[:, mt:mt + 1], op0=OP.mult, op1=OP.add)
            t2 = tmp.tile([P, CH], BF16, tag="scr")
            nc.vector.tensor_tensor(out=t2, in0=h[:, mt, chs], in1=rstd_b,
                                    op=OP.mult)
            nc.vector.scalar_tensor_tensor(
                out=h[:, mt, chs], in0=t2, scalar=g_t[:, mt:mt + 1],
                in1=c2, op0=OP.mult, op1=OP.add)


# ---------------- host side ----------------

def _pos_encoding_np():
    pos = np.arange(S, dtype=np.float32)[:, None]
    i = np.arange(D // 2, dtype=np.float32)[None, :]
    denom_s = np.power(np.float32(10000.0), (2.0 * i / D).astype(np.float32))
    denom_c = np.power(np.float32(10000.0),
                       (2.0 * (i + 1.0) / D).astype(np.float32))
    pe = np.zeros((S, D), np.float32)
    pe[:, 0::2] = np.sin(pos / denom_s)
    pe[:, 1::2] = np.cos(pos / denom_c)
    return pe  # [S, D]


def _prep_shared(emb, Wq, bq, Wk, bk, Wv, bv, Wo, bo, W1, b1, W2, b2,
                 g1, be1, g2, be2):
    f32 = np.float32
    scale = f32(1.0 / np.sqrt(DN))

    def cols(a, nt):  # [L, nt*128] -> [L, 128, nt]
        return np.ascontiguousarray(
            np.asarray(a).reshape(L, nt, P).transpose(0, 2, 1)).astype(f32)

    def pblocks(a):  # [L, D, D] -> [L, P, DT, DT*P]  (p=k-in-tile; kt,mt,m)
        return a.reshape(L, DT, P, DT, P).transpose(0, 2, 1, 3, 4).reshape(
            L, P, DT, DT * P)

    Wq, Wk, Wv, Wo = (np.asarray(a)[:L] for a in (Wq, Wk, Wv, Wo))
    W1, W2 = np.asarray(W1)[:L], np.asarray(W2)[:L]
    bq, bk, bv, bo = (np.asarray(a)[:L] for a in (bq, bk, bv, bo))
    b1, b2 = np.asarray(b1)[:L], np.asarray(b2)[:L]
    g1, be1, g2, be2 = (np.asarray(a)[:L] for a in (g1, be1, g2, be2))

    # [L, P, 4, DT, DT*P] : Wq*scale, Wk, Wo, Wv (concat-head layouts)
    wq_b = pblocks(Wq.transpose(0, 2, 1, 3).reshape(L, D, D) * scale)
    wk_b = pblocks(Wk.transpose(0, 2, 1, 3).reshape(L, D, D))
    wo_b = pblocks(Wo)
    # Wv: [l, p, kt, m] = Wv_cat[l, kt*128+p, m]
    wv_b = Wv.transpose(0, 2, 1, 3).reshape(L, DT, P, D).transpose(
        0, 2, 1, 3).reshape(L, P, DT, DT * P)
    wqkvo_h = np.ascontiguousarray(
        np.stack([wq_b, wk_b, wo_b, wv_b], axis=2)).astype(NPBF16)

    # wffn[l, p, m, 0:DT*P]: W1 block (kt, mm) at k-row p
    w1_b = W1.reshape(L, DT, P, FT, P).transpose(0, 2, 3, 1, 4).reshape(
        L, P, FT, DT * P)
    # wffn[l, p, m, DT*P + d]: W2[l, m*128+p, d]
    w2_b = W2.reshape(L, FT, P, D).transpose(0, 2, 1, 3)
    wffn_h = np.ascontiguousarray(
        np.concatenate([w1_b, w2_b], axis=3)).astype(NPBF16)

    pe_np = _pos_encoding_np()  # [S, D]
    pe_h = np.ascontiguousarray(
        pe_np.T.reshape(DT, P, S).transpose(1, 0, 2)).astype(NPBF16)

    return dict(
        emb=np.ascontiguousarray(emb).astype(NPBF16),
        pe=pe_h,
        wqkvo=wqkvo_h, wffn=wffn_h,
        bq_c=cols(bq.reshape(L, D) * scale, DT),
        bk_c=cols(bk.reshape(L, D), DT),
        bv_r=np.ascontiguousarray(bv.reshape(L, D)).astype(f32),
        bo_c=cols(bo, DT),
        b1_c=cols(b1, FT),
        b2_c=cols(b2, DT),
        g1_c=cols(g1, DT),
        g2_c=cols(g2, DT),
        ng1_c=cols(-g1, DT),
        ng2_c=cols(-g2, DT),
        be1_c=cols(be1, DT),
        be2_c=cols(be2, DT),
    )


def _make_in_maps(shared, x, padding_mask):
    x_i = np.asarray(x).astype(np.int32)
    mask_f = (1.0 - np.asarray(padding_mask).astype(np.float32)).astype(NPBF16)
    in_maps = []
    for c in range(NCORES):
        xs = x_i[c * BL:(c + 1) * BL].reshape(T)             # [1024]
        ms = mask_f[c * BL:(c + 1) * BL]                     # [2, 512, 512]
        # mmask[p, b, tci, s] = (1-mask)[b, s, tci*128+p]
        mt = np.ascontiguousarray(
            ms.transpose(0, 2, 1).reshape(BL, SC, P, S).transpose(2, 0, 1, 3))
        m = dict(shared)
        m["x_idx"] = np.ascontiguousarray(xs.reshape(TC, P))
        m["mmask"] = mt
        in_maps.append(m)
    return in_maps


def kernel(x, padding_mask, emb, Wq, bq, Wk, bk, Wv, bv, Wo, bo,
           W1, b1, W2, b2, g1, be1, g2, be2):
    if "nc" not in _PROGRAM_CACHE:
        _PROGRAM_CACHE["nc"] = _build_program()
    nc = _PROGRAM_CACHE["nc"]

    shared = _prep_shared(emb, Wq, bq, Wk, bk, Wv, bv, Wo, bo, W1, b1, W2, b2,
                          g1, be1, g2, be2)
    in_maps = _make_in_maps(shared, x, padding_mask)

    res = run_bass_kernel_spmd(nc, in_maps, core_ids=list(range(NCORES)))

    outs = []
    for c in range(NCORES):
        oc = np.asarray(res.results[c]["out"]).astype(np.float32)  # [P,DT,T]
        hc = oc.transpose(2, 1, 0).reshape(T, D)      # [T, D]
        outs.append(hc.reshape(BL, S, D))
    return np.concatenate(outs, axis=0).astype(np.float32)


if __name__ == "__main__":
    pass
